# revision 13
# baseline (speedup 1.0000x reference)
"""Cumulative mean along T (running mean) for input [8, 4096, 1024] f32.

out[b, t, f] = mean(x[b, :t+1, f])

Pure data parallel over batch: 8 cores, one batch element each.

fp16 end-to-end on the wire (host casts f32->fp16 on input, fp16->f32 on
output; tolerance is 2e-2 rel, fp16 quantization is ~5e-4): halves HBM/DMA
traffic vs f32.

Per core, T is processed in 16 superblocks of 256 timesteps = 128 PAIRS of
consecutive timesteps (partition p of superblock n holds t = 256n+2p and
256n+2p+1, i.e. 4 KiB of contiguous DRAM per partition per superblock -
every DMA packet is a full 4 KiB run).

The 1/(t+1) scale for the EVEN outputs is folded into the matmul
stationaries: per superblock n the triangular stationary is
lt_n[k,p] = r0[p,n]*[k<=p] and the carry-broadcast selector is
sel_n[31,p] = r0[p,n], so PSUM holds prefix*r0 directly. Then:
  - out0 = (x1 * -r0) + psum            one VectorE scalar_tensor_tensor,
                                        no prescale pass needed
  - out1 = psum * (r1/r0)               one ScalarE activation
  - chain hop: carry_{n+1} = carry_n + psum[96:128]*(256n+255)
                                        (un-scales row 127; VectorE stt
                                        with a float immediate)

Engine split: GpSimd pair sums S = x0+x1; TensorE 2 mains + 2 carry
broadcasts per superblock; VectorE hops + out0; ScalarE out1 + output
DMA issues (deferred one pipeline stage so they never head-of-line block
the ACT queue).

Software pipelining: superblocks in groups of 2; group g's broadcasts and
outputs are emitted after group g+1's mains; group g's output DMAs are
issued one further stage later. PSUM: 2 banks per superblock, 4
superblocks in flight = all 8 banks.
"""

import numpy as np

import concourse.bacc as bacc
import concourse.tile as tile
from concourse import mybir
from concourse.bass_utils import run_bass_kernel_spmd

B, T, F = 8, 4096, 1024
P = 128
SB = 256            # timesteps per superblock (128 pairs)
NSB = T // SB       # 16
FH = 512            # one PSUM bank of f32
NHALF = F // FH
CPG = 2             # superblocks per pipeline stage

F32 = mybir.dt.float32
F16 = mybir.dt.float16


def _build():
    nc = bacc.Bacc(None, target_bir_lowering=False)
    x_dram = nc.dram_tensor("x", [T, F], F16, kind="ExternalInput")
    out_dram = nc.dram_tensor("out", [T, F], F16, kind="ExternalOutput")

    # r0[p, n] = 1/(256n+2p+1) (even-t), r1[p, n] = 1/(256n+2p+2) (odd-t)
    tgrid = (np.arange(NSB)[None, :] * SB + 2 * np.arange(P)[:, None])
    r0_np = 1.0 / (tgrid + 1).astype(np.float64)
    r1_np = 1.0 / (tgrid + 2).astype(np.float64)

    # Per-superblock r0-scaled stationaries, packed side by side:
    # lt_all[k, n*128+p] = r0[p,n] for k<=p; sel_all[31, n*128+p] = r0[p,n].
    tri = np.triu(np.ones((P, P), dtype=np.float64))
    lt_all_np = (tri[:, None, :] * r0_np.T[None, :, :]).reshape(P, NSB * P)
    lt_all_np = np.ascontiguousarray(lt_all_np.astype(np.float16))
    sel_all_np = np.zeros((32, NSB * P), dtype=np.float16)
    sel_all_np[31, :] = r0_np.T.reshape(-1).astype(np.float16)

    r1_over_r0_np = (r1_np / r0_np).astype(np.float32)
    neg_r0_np = (-r0_np).astype(np.float32)

    lt_dram = nc.inline_tensor(lt_all_np, "lt_const")
    sel_dram = nc.inline_tensor(sel_all_np, "sel_const")
    rr_dram = nc.inline_tensor(
        np.ascontiguousarray(r1_over_r0_np), "rr_const"
    )
    nr0_dram = nc.inline_tensor(np.ascontiguousarray(neg_r0_np), "nr0_const")

    # t = n*256 + p*2 + q: partition p of superblock n holds the pair
    # (2p, 2p+1) as (q f) on the free axis -> 4 KiB contiguous per partition.
    x_pair = x_dram.rearrange("(n p q) f -> p n (q f)", p=P, q=2)
    out_pair = out_dram.rearrange("(n p q) f -> p n (q f)", p=P, q=2)

    with tile.TileContext(nc) as tc:
        with (
            tc.tile_pool(name="const", bufs=1) as cpool,
            tc.tile_pool(name="xin", bufs=8) as xpool,
            tc.tile_pool(name="xout", bufs=8) as opool,
            tc.tile_pool(name="spool", bufs=8) as spool,
            tc.tile_pool(name="run", bufs=8) as rpool,
            tc.tile_pool(name="psum", bufs=4, space="PSUM") as ppool,
        ):
            lt = cpool.tile([P, NSB * P], F16)
            nc.gpsimd.dma_start(lt[:], lt_dram[:])
            sel = cpool.tile([32, NSB * P], F16)
            nc.gpsimd.dma_start(sel[:], sel_dram[:])
            rr = cpool.tile([P, NSB], F32)
            nc.gpsimd.dma_start(rr[:], rr_dram[:])
            nr0 = cpool.tile([P, NSB], F32)
            nc.gpsimd.dma_start(nr0[:], nr0_dram[:])

            def flush(pend):
                psums, carries, x1s_, pbase = pend
                dmas = []
                for c in range(CPG):
                    n = pbase + c
                    # odd superblocks' broadcasts were emitted inline (the
                    # post-broadcast ACT hop needs them); only even ones here.
                    if carries[c] is not None and n % 2 == 0:
                        sstat = sel[:, n * P : (n + 1) * P]
                        for h in range(NHALF):
                            hs = slice(h * FH, (h + 1) * FH)
                            nc.tensor.matmul(
                                psums[c][:, hs], sstat, carries[c][:, hs],
                                start=False, stop=True,
                            )
                for c in range(CPG):
                    n = pbase + c
                    ot = opool.tile([P, 1, 2 * F], F16, tag="ot")
                    # odd outputs: out1[p] = psum[p] * (r1/r0)  (ScalarE)
                    nc.scalar.activation(
                        ot[:, 0, F : 2 * F], psums[c][:],
                        mybir.ActivationFunctionType.Identity,
                        scale=rr[:, n : n + 1],
                    )
                    # even outputs: out0[p] = x1[p]*(-r0) + psum[p] (VectorE)
                    nc.vector.scalar_tensor_tensor(
                        ot[:, 0, 0:F], x1s_[c], nr0[:, n : n + 1],
                        psums[c][:],
                        mybir.AluOpType.mult, mybir.AluOpType.add,
                    )
                    dmas.append((n, ot))
                return dmas

            def issue(dmas):
                for n, ot in dmas:
                    nc.scalar.dma_start(out_pair[:, n : n + 1, :], ot[:])

            carry = None  # [32, F] fp16; partition 31 = sum of sbs < n
            pend = None
            pend_dmas = None
            base = 0
            for g in range(NSB // CPG):
                xts = []
                for c in range(CPG):
                    n = base + c
                    xt = xpool.tile([P, 1, 2 * F], F16, tag="xt")
                    nc.sync.dma_start(xt[:], x_pair[:, n : n + 1, :])
                    xts.append(xt)

                psums = []
                carries = []
                x1s_ = []
                for c in range(CPG):
                    n = base + c
                    xt = xts[c]
                    x0 = xt[:, 0, 0:F]
                    x1 = xt[:, 0, F : 2 * F]
                    # pair sums on GpSimd (fp16, SBUF-only)
                    s = spool.tile([P, F], F16, tag="s")
                    nc.gpsimd.tensor_tensor(
                        s[:], x0, x1, mybir.AluOpType.add
                    )
                    ps = ppool.tile([P, F], F32)
                    psums.append(ps)
                    carries.append(carry)
                    x1s_.append(x1)
                    lstat = lt[:, n * P : (n + 1) * P]
                    if n < NSB - 1:
                        new_carry = rpool.tile([32, F], F16, tag="carry")
                    else:
                        new_carry = None
                    # Carry-chain hop, alternating executor so neither DVE
                    # nor ACT eats the whole chain:
                    #  - even n (VectorE): pre-broadcast rows 96..127,
                    #    carry' = psum*inv + carry, inv = 1/r0[127,n].
                    #  - odd n (ScalarE): the broadcast is emitted inline
                    #    right after the mains; post-broadcast row 127 is
                    #    r0*(S_total+carry), so carry' = psum*inv via a
                    #    plain Identity activation.
                    inv = float(SB * n + 2 * P - 1)
                    for h in range(NHALF):
                        hs = slice(h * FH, (h + 1) * FH)
                        nc.tensor.matmul(
                            ps[:, hs], lstat, s[:, hs],
                            start=True, stop=(n == 0),
                        )
                        if n % 2 == 0 and new_carry is not None:
                            if carry is None:
                                nc.vector.tensor_scalar_mul(
                                    new_carry[:, hs], ps[96:P, hs], inv
                                )
                            else:
                                nc.vector.scalar_tensor_tensor(
                                    new_carry[:, hs], ps[96:P, hs], inv,
                                    carry[:, hs],
                                    mybir.AluOpType.mult,
                                    mybir.AluOpType.add,
                                )
                    if n % 2 == 1:
                        # inline broadcast (closes the accumulation group)
                        sstat = sel[:, n * P : (n + 1) * P]
                        for h in range(NHALF):
                            hs = slice(h * FH, (h + 1) * FH)
                            nc.tensor.matmul(
                                ps[:, hs], sstat, carry[:, hs],
                                start=False, stop=True,
                            )
                            if new_carry is not None:
                                nc.scalar.activation(
                                    new_carry[:, hs], ps[96:P, hs],
                                    mybir.ActivationFunctionType.Identity,
                                    scale=inv,
                                )
                    if new_carry is not None:
                        carry = new_carry

                if pend_dmas is not None:
                    issue(pend_dmas)
                if pend is not None:
                    pend_dmas = flush(pend)
                pend = (psums, carries, x1s_, base)
                base += CPG

            issue(pend_dmas)
            issue(flush(pend))

    nc.compile()
    return nc


_NC_CACHE = None
last_results = None  # BassKernelResults of the most recent run (for test harness)


def kernel(inputs: np.ndarray) -> np.ndarray:
    global _NC_CACHE, last_results
    if _NC_CACHE is None:
        _NC_CACHE = _build()
    nc = _NC_CACHE
    x = np.asarray(inputs)
    assert x.shape == (B, T, F), x.shape
    x16 = np.ascontiguousarray(x.astype(np.float16))
    in_maps = [{"x": x16[b]} for b in range(B)]
    res = run_bass_kernel_spmd(nc, in_maps, core_ids=list(range(B)))
    last_results = res
    return np.stack(
        [r["out"].astype(np.float32) for r in res.results], axis=0
    )


# revision 17
# speedup vs baseline: 1.0669x; 1.0669x over previous
"""Cumulative mean along T (running mean) for input [8, 4096, 1024] f32.

out[b, t, f] = mean(x[b, :t+1, f])

Pure data parallel over batch: 8 cores, one batch element each.

fp16 end-to-end on the wire (host casts f32->fp16 on input, fp16->f32 on
output; tolerance is 2e-2 rel, fp16 quantization is ~5e-4): halves HBM/DMA
traffic vs f32.

Per core, T is processed in 16 superblocks of 256 timesteps = 128 PAIRS of
consecutive timesteps (partition p of superblock n holds t = 256n+2p and
256n+2p+1, i.e. 4 KiB of contiguous DRAM per partition per superblock -
every DMA packet is a full 4 KiB run).

The 1/(t+1) scale for the EVEN outputs is folded into the matmul
stationaries: per superblock n the triangular stationary is
lt_n[k,p] = r0[p,n]*[k<=p] and the carry-broadcast selector is
sel_n[31,p] = r0[p,n], so PSUM holds prefix*r0 directly. Then:
  - out0 = (x1 * -r0) + psum            one VectorE scalar_tensor_tensor,
                                        no prescale pass needed
  - out1 = psum * (r1/r0)               one ScalarE activation
  - chain hop: carry_{n+1} = carry_n + psum[96:128]*(256n+255)
                                        (un-scales row 127; VectorE stt
                                        with a float immediate)

Engine split: GpSimd pair sums S = x0+x1; TensorE 2 mains + 2 carry
broadcasts per superblock; VectorE hops + out0; ScalarE out1 + output
DMA issues (deferred one pipeline stage so they never head-of-line block
the ACT queue).

Software pipelining: superblocks in groups of 2; group g's broadcasts and
outputs are emitted after group g+1's mains; group g's output DMAs are
issued one further stage later. PSUM: 2 banks per superblock, 4
superblocks in flight = all 8 banks.
"""

import numpy as np

import concourse.bacc as bacc
import concourse.tile as tile
from concourse import mybir
from concourse.bass_utils import run_bass_kernel_spmd

B, T, F = 8, 4096, 1024
P = 128
SB = 256            # timesteps per superblock (128 pairs)
NSB = T // SB       # 16
FH = 512            # one PSUM bank of f32
NHALF = F // FH
CPG = 2             # superblocks per pipeline stage

F32 = mybir.dt.float32
F16 = mybir.dt.float16


def _build():
    nc = bacc.Bacc(None, target_bir_lowering=False)
    x_dram = nc.dram_tensor("x", [T, F], F16, kind="ExternalInput")
    out_dram = nc.dram_tensor("out", [T, F], F16, kind="ExternalOutput")

    # r0[p, n] = 1/(256n+2p+1) (even-t), r1[p, n] = 1/(256n+2p+2) (odd-t)
    tgrid = (np.arange(NSB)[None, :] * SB + 2 * np.arange(P)[:, None])
    r0_np = 1.0 / (tgrid + 1).astype(np.float64)
    r1_np = 1.0 / (tgrid + 2).astype(np.float64)

    # Per-superblock r0-scaled stationaries, packed side by side:
    # lt_all[k, n*128+p] = r0[p,n] for k<=p; sel_all[31, n*128+p] = r0[p,n].
    tri = np.triu(np.ones((P, P), dtype=np.float64))
    lt_all_np = (tri[:, None, :] * r0_np.T[None, :, :]).reshape(P, NSB * P)
    lt_all_np = np.ascontiguousarray(lt_all_np.astype(np.float16))
    sel_all_np = np.zeros((32, NSB * P), dtype=np.float16)
    sel_all_np[31, :] = r0_np.T.reshape(-1).astype(np.float16)

    r1_over_r0_np = (r1_np / r0_np).astype(np.float32)
    neg_r0_np = (-r0_np).astype(np.float32)

    lt_dram = nc.inline_tensor(lt_all_np, "lt_const")
    sel_dram = nc.inline_tensor(sel_all_np, "sel_const")
    rr_dram = nc.inline_tensor(
        np.ascontiguousarray(r1_over_r0_np), "rr_const"
    )
    nr0_dram = nc.inline_tensor(np.ascontiguousarray(neg_r0_np), "nr0_const")

    # t = n*256 + p*2 + q: partition p of superblock n holds the pair
    # (2p, 2p+1) as (q f) on the free axis -> 4 KiB contiguous per partition.
    x_pair = x_dram.rearrange("(n p q) f -> p n (q f)", p=P, q=2)
    out_pair = out_dram.rearrange("(n p q) f -> p n (q f)", p=P, q=2)

    with tile.TileContext(nc) as tc:
        with (
            tc.tile_pool(name="const", bufs=1) as cpool,
            tc.tile_pool(name="xin", bufs=8) as xpool,
            tc.tile_pool(name="xout", bufs=8) as opool,
            tc.tile_pool(name="spool", bufs=8) as spool,
            tc.tile_pool(name="run", bufs=8) as rpool,
            tc.tile_pool(name="psum", bufs=4, space="PSUM") as ppool,
        ):
            lt = cpool.tile([P, NSB * P], F16)
            nc.gpsimd.dma_start(lt[:], lt_dram[:])
            sel = cpool.tile([32, NSB * P], F16)
            nc.gpsimd.dma_start(sel[:], sel_dram[:])
            rr = cpool.tile([P, NSB], F32)
            nc.gpsimd.dma_start(rr[:], rr_dram[:])
            nr0 = cpool.tile([P, NSB], F32)
            nc.gpsimd.dma_start(nr0[:], nr0_dram[:])

            def flush(pend):
                psums, carries, x1s_, pbase = pend
                dmas = []
                for c in range(CPG):
                    n = pbase + c
                    if carries[c] is not None:
                        sstat = sel[:, n * P : (n + 1) * P]
                        for h in range(NHALF):
                            hs = slice(h * FH, (h + 1) * FH)
                            nc.tensor.matmul(
                                psums[c][:, hs], sstat, carries[c][:, hs],
                                start=False, stop=True,
                            )
                for c in range(CPG):
                    n = pbase + c
                    ot = opool.tile([P, 1, 2 * F], F16, tag="ot")
                    # odd outputs: out1[p] = psum[p] * (r1/r0)  (ScalarE)
                    nc.scalar.activation(
                        ot[:, 0, F : 2 * F], psums[c][:],
                        mybir.ActivationFunctionType.Identity,
                        scale=rr[:, n : n + 1],
                    )
                    # even outputs: out0[p] = x1[p]*(-r0) + psum[p] (VectorE)
                    nc.vector.scalar_tensor_tensor(
                        ot[:, 0, 0:F], x1s_[c], nr0[:, n : n + 1],
                        psums[c][:],
                        mybir.AluOpType.mult, mybir.AluOpType.add,
                    )
                    dmas.append((n, ot))
                return dmas

            def issue(dmas):
                for n, ot in dmas:
                    nc.scalar.dma_start(out_pair[:, n : n + 1, :], ot[:])

            carry = None  # [32, F] fp16; partition 31 = sum of sbs < n
            pend = None
            pend_dmas = None   # output DMAs deferred one stage
            pend_dmas2 = None  # ... and one more (input stream priority)
            base = 0
            for g in range(NSB // CPG):
                xts = []
                for c in range(CPG):
                    n = base + c
                    xt = xpool.tile([P, 1, 2 * F], F16, tag="xt")
                    nc.sync.dma_start(xt[:], x_pair[:, n : n + 1, :])
                    xts.append(xt)

                psums = []
                carries = []
                x1s_ = []
                for c in range(CPG):
                    n = base + c
                    xt = xts[c]
                    x0 = xt[:, 0, 0:F]
                    x1 = xt[:, 0, F : 2 * F]
                    # pair sums on GpSimd (fp16, SBUF-only)
                    s = spool.tile([P, F], F16, tag="s")
                    nc.gpsimd.tensor_tensor(
                        s[:], x0, x1, mybir.AluOpType.add
                    )
                    ps = ppool.tile([P, F], F32)
                    psums.append(ps)
                    carries.append(carry)
                    x1s_.append(x1)
                    lstat = lt[:, n * P : (n + 1) * P]
                    if n < NSB - 1:
                        new_carry = rpool.tile([32, F], F16, tag="carry")
                    else:
                        new_carry = None
                    # Hop per F-half (VectorE) interleaved with the mains;
                    # reads pre-broadcast rows 96..127 and un-scales row 127
                    # by 1/r0[127,n] = 256n+255.
                    inv = float(SB * n + 2 * P - 1)
                    for h in range(NHALF):
                        hs = slice(h * FH, (h + 1) * FH)
                        nc.tensor.matmul(
                            ps[:, hs], lstat, s[:, hs],
                            start=True, stop=(n == 0),
                        )
                        if new_carry is not None:
                            if carry is None:
                                nc.vector.tensor_scalar_mul(
                                    new_carry[:, hs], ps[96:P, hs], inv
                                )
                            else:
                                nc.vector.scalar_tensor_tensor(
                                    new_carry[:, hs], ps[96:P, hs], inv,
                                    carry[:, hs],
                                    mybir.AluOpType.mult,
                                    mybir.AluOpType.add,
                                )
                    if new_carry is not None:
                        carry = new_carry

                if pend_dmas2 is not None:
                    issue(pend_dmas2)
                pend_dmas2 = pend_dmas
                if pend is not None:
                    pend_dmas = flush(pend)
                pend = (psums, carries, x1s_, base)
                base += CPG

            issue(pend_dmas2)
            issue(pend_dmas)
            issue(flush(pend))

    nc.compile()
    return nc


_NC_CACHE = None
last_results = None  # BassKernelResults of the most recent run (for test harness)


def kernel(inputs: np.ndarray) -> np.ndarray:
    global _NC_CACHE, last_results
    if _NC_CACHE is None:
        _NC_CACHE = _build()
    nc = _NC_CACHE
    x = np.asarray(inputs)
    assert x.shape == (B, T, F), x.shape
    x16 = np.ascontiguousarray(x.astype(np.float16))
    in_maps = [{"x": x16[b]} for b in range(B)]
    res = run_bass_kernel_spmd(nc, in_maps, core_ids=list(range(B)))
    last_results = res
    return np.stack(
        [r["out"].astype(np.float32) for r in res.results], axis=0
    )


# revision 19
# speedup vs baseline: 1.1308x; 1.0599x over previous
"""Cumulative mean along T (running mean) for input [8, 4096, 1024] f32.

out[b, t, f] = mean(x[b, :t+1, f])

Pure data parallel over batch: 8 cores, one batch element each.

fp16 end-to-end on the wire (host casts f32->fp16 on input, fp16->f32 on
output; tolerance is 2e-2 rel, fp16 quantization is ~5e-4): halves HBM/DMA
traffic vs f32.

Per core, T is processed in 16 superblocks of 256 timesteps = 128 PAIRS of
consecutive timesteps (partition p of superblock n holds t = 256n+2p and
256n+2p+1, i.e. 4 KiB of contiguous DRAM per partition per superblock -
every DMA packet is a full 4 KiB run).

The 1/(t+1) scale for the EVEN outputs is folded into the matmul
stationaries: per superblock n the triangular stationary is
lt_n[k,p] = r0[p,n]*[k<=p] and the carry-broadcast selector is
sel_n[31,p] = r0[p,n], so PSUM holds prefix*r0 directly. Then:
  - out0 = (x1 * -r0) + psum            one VectorE scalar_tensor_tensor,
                                        no prescale pass needed
  - out1 = psum * (r1/r0)               one ScalarE activation
  - chain hop: carry_{n+1} = carry_n + psum[96:128]*(256n+255)
                                        (un-scales row 127; VectorE stt
                                        with a float immediate)

Engine split: GpSimd pair sums S = x0+x1; TensorE 2 mains + 2 carry
broadcasts per superblock; VectorE hops + out0; ScalarE out1 + output
DMA issues (deferred one pipeline stage so they never head-of-line block
the ACT queue).

Software pipelining: superblocks in groups of 2; group g's broadcasts and
outputs are emitted after group g+1's mains; group g's output DMAs are
issued one further stage later. PSUM: 2 banks per superblock, 4
superblocks in flight = all 8 banks.
"""

import numpy as np

import concourse.bacc as bacc
import concourse.tile as tile
from concourse import mybir
from concourse.bass_utils import run_bass_kernel_spmd

B, T, F = 8, 4096, 1024
P = 128
SB = 256            # timesteps per superblock (128 pairs)
NSB = T // SB       # 16
FH = 512            # one PSUM bank of f32
NHALF = F // FH
CPG = 2             # superblocks per pipeline stage

F32 = mybir.dt.float32
F16 = mybir.dt.float16


def _build():
    nc = bacc.Bacc(None, target_bir_lowering=False)
    x_dram = nc.dram_tensor("x", [T, F], F16, kind="ExternalInput")
    out_dram = nc.dram_tensor("out", [T, F], F16, kind="ExternalOutput")

    # r0[p, n] = 1/(256n+2p+1) (even-t), r1[p, n] = 1/(256n+2p+2) (odd-t)
    tgrid = (np.arange(NSB)[None, :] * SB + 2 * np.arange(P)[:, None])
    r0_np = 1.0 / (tgrid + 1).astype(np.float64)
    r1_np = 1.0 / (tgrid + 2).astype(np.float64)

    # Per-superblock r0-scaled stationaries, packed side by side:
    # lt_all[k, n*128+p] = r0[p,n] for k<=p; sel_all[31, n*128+p] = r0[p,n].
    tri = np.triu(np.ones((P, P), dtype=np.float64))
    lt_all_np = (tri[:, None, :] * r0_np.T[None, :, :]).reshape(P, NSB * P)
    lt_all_np = np.ascontiguousarray(lt_all_np.astype(np.float16))
    sel_all_np = np.zeros((32, NSB * P), dtype=np.float16)
    sel_all_np[31, :] = r0_np.T.reshape(-1).astype(np.float16)

    r1_over_r0_np = (r1_np / r0_np).astype(np.float32)
    neg_r0_np = (-r0_np).astype(np.float32)

    lt_dram = nc.inline_tensor(lt_all_np, "lt_const")
    sel_dram = nc.inline_tensor(sel_all_np, "sel_const")
    rr_dram = nc.inline_tensor(
        np.ascontiguousarray(r1_over_r0_np), "rr_const"
    )
    nr0_dram = nc.inline_tensor(np.ascontiguousarray(neg_r0_np), "nr0_const")

    # t = n*256 + p*2 + q: partition p of superblock n holds the pair
    # (2p, 2p+1) as (q f) on the free axis -> 4 KiB contiguous per partition.
    x_pair = x_dram.rearrange("(n p q) f -> p n (q f)", p=P, q=2)
    out_pair = out_dram.rearrange("(n p q) f -> p n (q f)", p=P, q=2)

    with tile.TileContext(nc) as tc:
        with (
            tc.tile_pool(name="const", bufs=1) as cpool,
            tc.tile_pool(name="xin", bufs=8) as xpool,
            tc.tile_pool(name="xout", bufs=8) as opool,
            tc.tile_pool(name="spool", bufs=8) as spool,
            tc.tile_pool(name="run", bufs=8) as rpool,
            tc.tile_pool(name="psum", bufs=4, space="PSUM") as ppool,
        ):
            lt = cpool.tile([P, NSB * P], F16)
            nc.gpsimd.dma_start(lt[:], lt_dram[:])
            sel = cpool.tile([32, NSB * P], F16)
            nc.gpsimd.dma_start(sel[:], sel_dram[:])
            rr = cpool.tile([P, NSB], F32)
            nc.gpsimd.dma_start(rr[:], rr_dram[:])
            nr0 = cpool.tile([P, NSB], F32)
            nc.gpsimd.dma_start(nr0[:], nr0_dram[:])

            def flush(pend):
                psums, carries, x1s_, pbase = pend
                dmas = []
                for c in range(CPG):
                    n = pbase + c
                    if carries[c] is not None:
                        sstat = sel[:, n * P : (n + 1) * P]
                        for h in range(NHALF):
                            hs = slice(h * FH, (h + 1) * FH)
                            nc.tensor.matmul(
                                psums[c][:, hs], sstat, carries[c][:, hs],
                                start=False, stop=True,
                            )
                for c in range(CPG):
                    n = pbase + c
                    ot = opool.tile([P, 1, 2 * F], F16, tag="ot")
                    # odd outputs: out1[p] = psum[p] * (r1/r0)  (ScalarE)
                    nc.scalar.activation(
                        ot[:, 0, F : 2 * F], psums[c][:],
                        mybir.ActivationFunctionType.Identity,
                        scale=rr[:, n : n + 1],
                    )
                    # even outputs: out0[p] = x1[p]*(-r0) + psum[p] (VectorE)
                    nc.vector.scalar_tensor_tensor(
                        ot[:, 0, 0:F], x1s_[c], nr0[:, n : n + 1],
                        psums[c][:],
                        mybir.AluOpType.mult, mybir.AluOpType.add,
                    )
                    dmas.append((n, ot))
                return dmas

            def issue(dmas):
                for n, ot in dmas:
                    nc.scalar.dma_start(out_pair[:, n : n + 1, :], ot[:])

            carry = None  # [32, F] fp16; partition 31 = sum of sbs < n
            pend = None
            pend_dmas = None   # output DMAs deferred one stage
            pend_dmas2 = None  # ... and one more (input stream priority)
            base = 0
            for g in range(NSB // CPG):
                xts = []
                for c in range(CPG):
                    n = base + c
                    xt = xpool.tile([P, 1, 2 * F], F16, tag="xt")
                    nc.sync.dma_start(xt[:], x_pair[:, n : n + 1, :])
                    xts.append(xt)

                psums = []
                carries = []
                x1s_ = []
                for c in range(CPG):
                    n = base + c
                    xt = xts[c]
                    x0 = xt[:, 0, 0:F]
                    x1 = xt[:, 0, F : 2 * F]
                    # pair sums on GpSimd (fp16, SBUF-only)
                    s = spool.tile([P, F], F16, tag="s")
                    nc.gpsimd.tensor_tensor(
                        s[:], x0, x1, mybir.AluOpType.add
                    )
                    ps = ppool.tile([P, F], F32)
                    psums.append(ps)
                    carries.append(carry)
                    x1s_.append(x1)
                    lstat = lt[:, n * P : (n + 1) * P]
                    if n < NSB - 1:
                        new_carry = rpool.tile([32, F], F16, tag="carry")
                    else:
                        new_carry = None
                    # Mains per F-half; single full-width hop (VectorE)
                    # right after: reads pre-broadcast rows 96..127 and
                    # un-scales row 127 by 1/r0[127,n] = 256n+255.
                    inv = float(SB * n + 2 * P - 1)
                    for h in range(NHALF):
                        hs = slice(h * FH, (h + 1) * FH)
                        nc.tensor.matmul(
                            ps[:, hs], lstat, s[:, hs],
                            start=True, stop=(n == 0),
                        )
                    if new_carry is not None:
                        if carry is None:
                            nc.vector.tensor_scalar_mul(
                                new_carry[:], ps[96:P, :], inv
                            )
                        else:
                            nc.vector.scalar_tensor_tensor(
                                new_carry[:], ps[96:P, :], inv,
                                carry[:],
                                mybir.AluOpType.mult,
                                mybir.AluOpType.add,
                            )
                        carry = new_carry

                if pend_dmas is not None:
                    issue(pend_dmas)
                if pend is not None:
                    pend_dmas = flush(pend)
                pend = (psums, carries, x1s_, base)
                base += CPG

            issue(pend_dmas)
            issue(flush(pend))

    nc.compile()
    return nc


_NC_CACHE = None
last_results = None  # BassKernelResults of the most recent run (for test harness)


def kernel(inputs: np.ndarray) -> np.ndarray:
    global _NC_CACHE, last_results
    if _NC_CACHE is None:
        _NC_CACHE = _build()
    nc = _NC_CACHE
    x = np.asarray(inputs)
    assert x.shape == (B, T, F), x.shape
    x16 = np.ascontiguousarray(x.astype(np.float16))
    in_maps = [{"x": x16[b]} for b in range(B)]
    res = run_bass_kernel_spmd(nc, in_maps, core_ids=list(range(B)))
    last_results = res
    return np.stack(
        [r["out"].astype(np.float32) for r in res.results], axis=0
    )
